# revision 1
# baseline (speedup 1.0000x reference)
"""Trainium2 Bass kernel for nn_Experts (topk_masking).

Math (reference):
  R = concat(h,us,ue) @ W_r.T + b_r                       [1,1,512]
  x = concat(u, R.broadcast)                              [1,S,1536]
  h1 = (x @ W_nn.T + b_nn).reshape(S,512,16)
  h2 = (x @ W_no.T + b_no).reshape(S,512,16) * noise
  g  = top2-masked softmax over experts of (h1+h2)
  e  = (x @ W_E.T + b_E).reshape(S,512,16)
  out = (g*e).mean(-1)                                    [1,S,512]

Sharding: the NE*DIM output-feature dim of the three projections is sharded
across 8 cores (64 dims x 16 experts each, contiguous feature slice). The
token-independent R-path is folded into a per-feature constant c[f] computed
once per core, so the per-token matmuls contract only over u's 1024 features.

Precision: gating matmuls use a 2-term fp32 split (11 explicit mantissa bits
+ residual) through the PE's float32r mode (verified: <=11-bit operands pass
through exactly), plus a bf16 cross-term; this lands the gating logits at
~fp32 accuracy so top-2 selection matches the fp32 reference. The e-matmul
runs in bf16 (smooth, no selection discontinuity).
"""
import numpy as np
import ml_dtypes

DIM = 512
NE = 16
S = 4096
KU = 2 * DIM        # u features = 1024
KR = DIM            # R features = 512
KX = 5 * DIM        # concat(h,us,ue) = 2560
NCORES = 8
DL = DIM // NCORES  # 64 dims per core
FL = DL * NE        # 1024 features per core
MCH = S // 128      # 32 token chunks

_MASK11 = np.uint32(0xFFFFF000)  # keep 11 explicit mantissa bits

TRACE = False
DEBUG = False
_CACHE = {}


def _trunc11(a):
    a = np.ascontiguousarray(a, dtype=np.float32)
    return (a.view(np.uint32) & _MASK11).view(np.float32)


def _build():
    import concourse.bass as bass
    import concourse.mybir as mybir
    import concourse.tile as tile
    from concourse import bacc
    from contextlib import ExitStack

    F32 = mybir.dt.float32
    F32R = mybir.dt.float32r
    BF16 = mybir.dt.bfloat16
    U32 = mybir.dt.uint32
    AX = mybir.AxisListType
    OP = mybir.AluOpType
    ACTF = mybir.ActivationFunctionType

    nc = bacc.Bacc("TRN2", target_bir_lowering=False, debug=False,
                   num_devices=NCORES)

    def dram(name, shape, dt, kind="ExternalInput"):
        return nc.dram_tensor(name, shape, dt, kind=kind)

    # per-core inputs (same names on every core; data differs per core)
    uhT = dram("uhT", [KU, S], F32R)
    ulT = dram("ulT", [KU, S], F32R)
    u8T = dram("u8T", [KU, S], BF16)
    whnnT = dram("whnnT", [KU, FL], F32R)
    whnoT = dram("whnoT", [KU, FL], F32R)
    wl8T = dram("wl8T", [KU, 2 * FL], BF16)   # [:, :FL]=nn resid, [:, FL:]=no resid
    we8T = dram("we8T", [KU, FL], BF16)
    noise_c = dram("noise_c", [S, FL], F32)
    hxf = dram("hxf", [KX], F32)
    wrT = dram("wrT", [KX, KR], F32)
    b_r = dram("b_r", [KR], F32)
    wRh_nn = dram("wRh_nn", [KR, FL], F32R)
    wRl_nn = dram("wRl_nn", [KR, FL], F32R)
    wRh_no = dram("wRh_no", [KR, FL], F32R)
    wRl_no = dram("wRl_no", [KR, FL], F32R)
    wR_E = dram("wR_E", [KR, FL], F32R)
    bias_c = dram("bias_c", [3 * FL], F32)
    out_c = dram("out_c", [S, DL], F32, kind="ExternalOutput")
    dbg = {}
    if DEBUG:
        for nm in ["h1", "h2", "e", "m", "q", "mask"]:
            dbg[nm] = dram("dbg_" + nm, [128, FL], F32, kind="ExternalOutput")
        for nm in ["v1", "v2", "s"]:
            dbg[nm] = dram("dbg_" + nm, [128, DL], F32, kind="ExternalOutput")
        dbg["cc"] = dram("dbg_cc", [2, 3 * FL], F32, kind="ExternalOutput")
        dbg["R"] = dram("dbg_R", [128, 4], F32, kind="ExternalOutput")

    with tile.TileContext(nc) as tc, ExitStack() as ctx:
        wpool = ctx.enter_context(tc.tile_pool(name="w", bufs=1))

        # resident weights (one big DMA each)
        whnn_t = wpool.tile([128, 8, FL], F32R)
        whno_t = wpool.tile([128, 8, FL], F32R)
        wl8_t = wpool.tile([128, 8, 2 * FL], BF16)
        we8_t = wpool.tile([128, 8, FL], BF16)
        nc.sync.dma_start(whnn_t[:], whnnT.ap().rearrange("(kc p) f -> p kc f", p=128))
        nc.sync.dma_start(whno_t[:], whnoT.ap().rearrange("(kc p) f -> p kc f", p=128))
        nc.sync.dma_start(wl8_t[:], wl8T.ap().rearrange("(kc p) f -> p kc f", p=128))
        nc.sync.dma_start(we8_t[:], we8T.ap().rearrange("(kc p) f -> p kc f", p=128))

        # survives the whole kernel: bias/R constant rows + ones for the K=2 matmul
        ccsb = wpool.tile([2, 3 * FL], F32R)
        onesf = wpool.tile([2, 128], F32)
        nc.vector.memset(onesf[:], 1.0)
        ones2 = wpool.tile([2, 128], F32R)
        nc.vector.tensor_copy(ones2[:], onesf[:])

        # ---------------- stage 0: R then c ----------------
        with ExitStack() as s0:
            s0sb = s0.enter_context(tc.tile_pool(name="s0sb", bufs=1))
            s0rot = s0.enter_context(tc.tile_pool(name="s0rot", bufs=4))
            s0ps = s0.enter_context(tc.tile_pool(name="s0ps", bufs=1, space="PSUM"))

            hx_t = s0sb.tile([128, 20], F32)
            nc.sync.dma_start(hx_t[:], hxf.ap().rearrange("(kc p) -> p kc", p=128))

            # R = hx @ W_r.T with W_r stationary: out lands as [128, 4]
            # across partitions directly (R[mo*128+p] = psR[p, mo]).
            # NOTE: start=True clears has_written for the whole PSUM bank, so
            # each mo's accumulation chain needs its own bank.
            psR = []
            for mo in range(4):
                psR_mo = s0ps.tile([128, 1], F32, tag=f"psR{mo}")
                psR.append(psR_mo)
            for kc in range(20):
                ksl = slice(kc * 128, (kc + 1) * 128)
                wr_ch = s0rot.tile([128, KR], F32, tag="rotf")
                nc.sync.dma_start(wr_ch[:], wrT.ap()[ksl, :])
                for mo in range(4):
                    msl = slice(mo * 128, (mo + 1) * 128)
                    nc.tensor.matmul(psR[mo][:], wr_ch[:, msl],
                                     hx_t[:, kc:kc + 1],
                                     start=(kc == 0), stop=(kc == 19))

            brt = s0sb.tile([128, 4], F32)
            nc.sync.dma_start(brt[:], b_r.ap().rearrange("(mo p) -> p mo", p=128))
            Rcol = s0sb.tile([128, 4], F32)
            for mo in range(4):
                nc.vector.tensor_add(Rcol[:, mo:mo + 1], psR[mo][:],
                                     brt[:, mo:mo + 1])

            Rh = s0sb.tile([128, 4], F32)
            nc.vector.tensor_scalar(Rh[:].bitcast(U32), Rcol[:].bitcast(U32),
                                    int(_MASK11), None, OP.bitwise_and)
            Rl = s0sb.tile([128, 4], F32)
            nc.vector.tensor_sub(Rl[:], Rcol[:], Rh[:])
            # broadcast along the stationary M dim (value replicated per token)
            Rbch = s0sb.tile([128, 4, 128], F32R)
            nc.vector.tensor_copy(Rbch[:], Rh[:].broadcast_to([128, 4, 128]))
            Rbcl = s0sb.tile([128, 4, 128], F32R)
            nc.vector.tensor_copy(Rbcl[:], Rl[:].broadcast_to([128, 4, 128]))
            if DEBUG:
                nc.sync.dma_start(dbg["R"].ap(), Rcol[:])

            # c pieces: piece 0 -> c_nn, 1 -> c_no, 2 -> c_E (each FL wide)
            # biasb2 doubles as the c+bias staging buffer (updated in place);
            # all DVE work stays on partition 0 (engines need lane-0 alignment)
            biasb2 = s0sb.tile([1, 3 * FL], F32)
            nc.sync.dma_start(biasb2[:],
                              bias_c.ap().rearrange("(o f) -> o f", o=1))
            cpsum = s0ps.tile([128, FL], F32, tag="cps")
            pieces = [(wRh_nn, wRl_nn), (wRh_no, wRl_no), (wR_E, None)]
            for pi, (wh_d, wl_d) in enumerate(pieces):
                for kc in range(4):
                    ksl = slice(kc * 128, (kc + 1) * 128)
                    for half in range(2):
                        fsl = slice(half * 512, (half + 1) * 512)
                        whch = s0rot.tile([128, 512], F32R, tag="rot")
                        nc.sync.dma_start(whch[:], wh_d.ap()[ksl, fsl])
                        nc.tensor.matmul(cpsum[:, fsl], Rbch[:, kc, :], whch[:],
                                         start=(kc == 0), stop=False)
                        nc.tensor.matmul(cpsum[:, fsl], Rbcl[:, kc, :], whch[:],
                                         start=False, stop=False)
                        if wl_d is not None:
                            wlch = s0rot.tile([128, 512], F32R, tag="rot")
                            nc.sync.dma_start(wlch[:], wl_d.ap()[ksl, fsl])
                            nc.tensor.matmul(cpsum[:, fsl], Rbch[:, kc, :],
                                             wlch[:], start=False,
                                             stop=(kc == 3))
                        elif kc == 3:
                            nc.tensor.matmul(cpsum[:, fsl], Rbcl[:, kc, :],
                                             whch[:], start=False, stop=True)
                psl = slice(pi * FL, (pi + 1) * FL)
                nc.vector.tensor_add(biasb2[0:1, psl], cpsum[0:1, :],
                                     biasb2[0:1, psl])

            # split c into 11-bit head + residual, round both to f32r on
            # partition 0, then DMA into the two rows of ccsb
            cht = s0sb.tile([1, 3 * FL], F32)
            nc.vector.tensor_scalar(cht[0:1, :].bitcast(U32),
                                    biasb2[0:1, :].bitcast(U32),
                                    int(_MASK11), None, OP.bitwise_and)
            clt = s0sb.tile([1, 3 * FL], F32)
            nc.vector.tensor_sub(clt[0:1, :], biasb2[0:1, :], cht[0:1, :])
            chr_ = s0sb.tile([1, 3 * FL], F32R)
            nc.vector.tensor_copy(chr_[0:1, :], cht[0:1, :])
            clr_ = s0sb.tile([1, 3 * FL], F32R)
            nc.vector.tensor_copy(clr_[0:1, :], clt[0:1, :])
            nc.sync.dma_start(ccsb[0:1, :], chr_[0:1, :])
            nc.sync.dma_start(ccsb[1:2, :], clr_[0:1, :])
            if DEBUG:
                nc.sync.dma_start(dbg["cc"].ap()[0:1, :], cht[0:1, :])
                nc.sync.dma_start(dbg["cc"].ap()[1:2, :], clt[0:1, :])

        # ---------------- main loop over 32 token chunks ----------------
        spool = ctx.enter_context(tc.tile_pool(name="stream", bufs=2))
        epool = ctx.enter_context(tc.tile_pool(name="epi", bufs=1))
        mpsum = ctx.enter_context(tc.tile_pool(name="mps", bufs=1, space="PSUM"))

        uhT_r = uhT.ap().rearrange("(kc p) t -> p kc t", p=128)
        ulT_r = ulT.ap().rearrange("(kc p) t -> p kc t", p=128)
        u8T_r = u8T.ap().rearrange("(kc p) t -> p kc t", p=128)

        for m in range(MCH):
            tsl = slice(m * 128, (m + 1) * 128)
            xh_t = spool.tile([128, 8, 128], F32R, tag="xh")
            xl_t = spool.tile([128, 8, 128], F32R, tag="xl")
            x8_t = spool.tile([128, 8, 128], BF16, tag="x8")
            nz_t = spool.tile([128, FL], F32, tag="nz")
            nc.sync.dma_start(xh_t[:], uhT_r[:, :, tsl])
            nc.sync.dma_start(xl_t[:], ulT_r[:, :, tsl])
            nc.sync.dma_start(x8_t[:], u8T_r[:, :, tsl])
            nc.sync.dma_start(nz_t[:], noise_c.ap()[tsl, :])

            h1p = mpsum.tile([128, FL], F32, tag="h1")
            h2p = mpsum.tile([128, FL], F32, tag="h2")
            ep = mpsum.tile([128, FL], F32, tag="e")

            for k in range(8):
                lh = xh_t[:, k, :]
                ll = xl_t[:, k, :]
                l8 = x8_t[:, k, :]
                st = (k == 0)
                for half in range(2):
                    fsl = slice(half * 512, (half + 1) * 512)
                    # stationary xh: main gating terms
                    nc.tensor.matmul(h2p[:, fsl], lh, whno_t[:, k, fsl],
                                     start=st, stop=False)
                    nc.tensor.matmul(h1p[:, fsl], lh, whnn_t[:, k, fsl],
                                     start=st, stop=False)
                for half in range(2):
                    fsl = slice(half * 512, (half + 1) * 512)
                    # stationary xl: residual-x terms
                    nc.tensor.matmul(h2p[:, fsl], ll, whno_t[:, k, fsl],
                                     start=False, stop=False)
                    nc.tensor.matmul(h1p[:, fsl], ll, whnn_t[:, k, fsl],
                                     start=False, stop=False)
                for half in range(2):
                    fsl = slice(half * 512, (half + 1) * 512)
                    fsl_no = slice(FL + half * 512, FL + (half + 1) * 512)
                    # stationary x8 (bf16): residual-W cross terms + e matmul
                    nc.tensor.matmul(h2p[:, fsl], l8, wl8_t[:, k, fsl_no],
                                     start=False, stop=False)
                    nc.tensor.matmul(h1p[:, fsl], l8, wl8_t[:, k, fsl],
                                     start=False, stop=False)
                    nc.tensor.matmul(ep[:, fsl], l8, we8_t[:, k, fsl],
                                     start=st, stop=False)

            # bias + R-path constant via K=2 ones-matmul (rows: c_head, c_resid)
            for half in range(2):
                fsl = slice(half * 512, (half + 1) * 512)
                nc.tensor.matmul(h1p[:, fsl], ones2[:], ccsb[:, fsl],
                                 start=False, stop=True)
                nc.tensor.matmul(h2p[:, fsl], ones2[:],
                                 ccsb[:, FL + half * 512:FL + (half + 1) * 512],
                                 start=False, stop=True)
                nc.tensor.matmul(ep[:, fsl], ones2[:],
                                 ccsb[:, 2 * FL + half * 512:2 * FL + (half + 1) * 512],
                                 start=False, stop=True)

            # ---------------- epilogue ----------------
            if DEBUG and m == 0:
                for nm, src in [("h1", h1p), ("h2", h2p), ("e", ep)]:
                    dtmp = epool.tile([128, FL], F32, tag="dbg" + nm)
                    nc.scalar.copy(dtmp[:], src[:])
                    nc.sync.dma_start(dbg[nm].ap(), dtmp[:])
            t_t = epool.tile([128, FL], F32, tag="t")
            nc.vector.tensor_mul(t_t[:], h2p[:], nz_t[:])
            m_t = epool.tile([128, FL], F32, tag="m")
            nc.vector.tensor_add(m_t[:], t_t[:], h1p[:])

            mg = m_t[:].rearrange("p (d e) -> p d e", e=NE)
            v1 = epool.tile([128, DL], F32, tag="v1")
            nc.vector.tensor_reduce(v1[:], mg, AX.X, op=OP.max)
            eq1 = epool.tile([128, FL], F32, tag="eq1")
            nc.vector.tensor_tensor(eq1[:].rearrange("p (d e) -> p d e", e=NE),
                                    mg, v1[:].broadcast_to([128, DL, NE]),
                                    OP.is_equal)
            m2 = epool.tile([128, FL], F32, tag="m2")
            nc.vector.scalar_tensor_tensor(m2[:], eq1[:], -1e30, m_t[:],
                                           OP.mult, OP.add)
            v2 = epool.tile([128, DL], F32, tag="v2")
            nc.vector.tensor_reduce(v2[:], m2[:].rearrange("p (d e) -> p d e", e=NE),
                                    AX.X, op=OP.max)
            mask = epool.tile([128, FL], F32, tag="mask")
            nc.vector.tensor_tensor(mask[:].rearrange("p (d e) -> p d e", e=NE),
                                    mg, v2[:].broadcast_to([128, DL, NE]),
                                    OP.is_ge)
            q = epool.tile([128, FL], F32, tag="q")
            nc.scalar.activation(q[:], m_t[:], ACTF.Exp)

            t1 = epool.tile([128, FL], F32, tag="t1")
            nc.vector.tensor_mul(t1[:], mask[:], ep[:])
            t2 = epool.tile([128, FL], F32, tag="t2")
            nc.vector.tensor_mul(t2[:], t1[:], q[:])
            s_t = epool.tile([128, DL], F32, tag="s")
            nc.vector.tensor_reduce(s_t[:], t2[:].rearrange("p (d e) -> p d e", e=NE),
                                    AX.X, op=OP.add)

            if DEBUG and m == 0:
                for nm, src in [("m", m_t), ("q", q), ("mask", mask)]:
                    nc.sync.dma_start(dbg[nm].ap(), src[:])
                for nm, src in [("v1", v1), ("v2", v2), ("s", s_t)]:
                    nc.sync.dma_start(dbg[nm].ap(), src[:])
            ev12 = epool.tile([128, 2 * DL], F32, tag="ev12")
            nc.scalar.activation(ev12[:, :DL], v1[:], ACTF.Exp)
            nc.scalar.activation(ev12[:, DL:], v2[:], ACTF.Exp)
            z_t = epool.tile([128, DL], F32, tag="z")
            nc.vector.tensor_add(z_t[:], ev12[:, :DL], ev12[:, DL:])
            r_t = epool.tile([128, DL], F32, tag="r")
            nc.vector.reciprocal(r_t[:], z_t[:])
            o_t = epool.tile([128, DL], F32, tag="o")
            nc.vector.scalar_tensor_tensor(o_t[:], s_t[:], 1.0 / NE, r_t[:],
                                           OP.mult, OP.mult)
            nc.sync.dma_start(out_c.ap()[tsl, :], o_t[:])

    nc.compile()
    return nc


def _get_program():
    if "nc" not in _CACHE:
        _CACHE["nc"] = _build()
    return _CACHE["nc"]


def kernel(h, us, ue, u, noise, W_nn, b_nn, W_no, b_no, W_E, b_E, W_r, b_r):
    from concourse.bass_utils import run_bass_kernel_spmd

    f32 = np.float32
    bf16 = ml_dtypes.bfloat16
    u2 = np.ascontiguousarray(np.asarray(u, dtype=f32).reshape(S, KU))
    uh = _trunc11(u2)
    ul = (u2 - uh).astype(f32)
    uhT = np.ascontiguousarray(uh.T)
    ulT = np.ascontiguousarray(ul.T)
    u8T = np.ascontiguousarray(u2.T.astype(bf16))

    hx = np.concatenate([np.asarray(h, dtype=f32).ravel(),
                         np.asarray(us, dtype=f32).ravel(),
                         np.asarray(ue, dtype=f32).ravel()]).astype(f32)
    W_r = np.asarray(W_r, dtype=f32)
    wrT = np.ascontiguousarray(W_r.T)
    b_r = np.ascontiguousarray(np.asarray(b_r, dtype=f32))

    W_nn = np.asarray(W_nn, dtype=f32)
    W_no = np.asarray(W_no, dtype=f32)
    W_E = np.asarray(W_E, dtype=f32)
    b_nn = np.asarray(b_nn, dtype=f32)
    b_no = np.asarray(b_no, dtype=f32)
    b_E = np.asarray(b_E, dtype=f32)
    noise4 = np.asarray(noise, dtype=f32).reshape(S, DIM, NE)

    in_maps = []
    for c in range(NCORES):
        fsl = slice(c * FL, (c + 1) * FL)
        wnn_u = W_nn[fsl, :KU]
        wno_u = W_no[fsl, :KU]
        wE_u = W_E[fsl, :KU]
        wnn_h = _trunc11(wnn_u)
        wno_h = _trunc11(wno_u)
        wl8 = np.concatenate([(wnn_u - wnn_h).T.astype(bf16),
                              (wno_u - wno_h).T.astype(bf16)], axis=1)
        im = {
            "uhT": uhT, "ulT": ulT, "u8T": u8T,
            "whnnT": np.ascontiguousarray(wnn_h.T),
            "whnoT": np.ascontiguousarray(wno_h.T),
            "wl8T": np.ascontiguousarray(wl8),
            "we8T": np.ascontiguousarray(wE_u.T.astype(bf16)),
            "noise_c": np.ascontiguousarray(
                noise4[:, c * DL:(c + 1) * DL, :].reshape(S, FL)),
            "hxf": hx, "wrT": wrT, "b_r": b_r,
            "wRh_nn": np.ascontiguousarray(_trunc11(W_nn[fsl, KU:]).T),
            "wRl_nn": np.ascontiguousarray(
                (W_nn[fsl, KU:] - _trunc11(W_nn[fsl, KU:])).T.astype(f32)),
            "wRh_no": np.ascontiguousarray(_trunc11(W_no[fsl, KU:]).T),
            "wRl_no": np.ascontiguousarray(
                (W_no[fsl, KU:] - _trunc11(W_no[fsl, KU:])).T.astype(f32)),
            "wR_E": np.ascontiguousarray(W_E[fsl, KU:].T.astype(f32)),
            "bias_c": np.concatenate([b_nn[fsl], b_no[fsl], b_E[fsl]]).astype(f32),
        }
        in_maps.append(im)

    nc = _get_program()
    res = run_bass_kernel_spmd(nc, in_maps, core_ids=list(range(NCORES)),
                               trace=TRACE)
    _CACHE["last_results"] = res
    out = np.empty((1, S, DIM), dtype=f32)
    for c in range(NCORES):
        out[0, :, c * DL:(c + 1) * DL] = res.results[c]["out_c"]
    return out



# revision 42
# speedup vs baseline: 2.4594x; 2.4594x over previous
"""Trainium2 Bass kernel for nn_Experts (topk_masking).

Math (reference):
  R = concat(h,us,ue) @ W_r.T + b_r                       [1,1,512]
  x = concat(u, R.broadcast)                              [1,S,1536]
  h1 = (x @ W_nn.T + b_nn).reshape(S,512,16)
  h2 = (x @ W_no.T + b_no).reshape(S,512,16) * noise
  g  = top2-masked softmax over experts of (h1+h2)
  e  = (x @ W_E.T + b_E).reshape(S,512,16)
  out = (g*e).mean(-1)                                    [1,S,512]

Sharding: the NE*DIM output-feature dim of the three projections is sharded
across 8 cores (64 dims x 16 experts each, contiguous feature slice).

The R-dependent part of each projection is a per-feature constant c[f]
(token-independent); it is computed on the host in fp64, shipped as an
11-bit head + residual pair, broadcast across partitions once via a K=2
ones-matmul, and folded into the logits on the Pool engine.

Precision: the PE's float32r mode rounds each operand to 11 explicit
mantissa bits (RNE) internally, so a single f32r pass per projection gives
logits accurate to ~1.5e-4 relative.  Top-2 selection flips where the
expert margin is below that, which lands total rel_l2 ~1.16e-2 against the
fp32 reference (gate 2e-2).  e is also one f32r pass; the value path
(q=exp(m), e, their products) runs in bf16 (DVE 2x mode), selection in f32.

Engine split per 128-token x 512-feature half-iteration:
  PE   24 f32r matmuls (3 projections x 8 K-chunks)          ~5.1us
  Act  3 PSUM->SBUF drains + exp(m) + exp(v1|v2)             ~2.8us
  Pool +c folds and the noise multiply (SBUF only)           ~4.4us
  DVE  f32 top-2 selection chain + bf16 value path + reduces ~5.1us
Feature-half 0 of all chunks runs first so the main loop starts after
only half the weight bytes; weights stream as per-(proj,half,k) pieces.
"""
import numpy as np

DIM = 512
NE = 16
S = 4096
KU = 2 * DIM        # u features = 1024
NCORES = 8
DL = DIM // NCORES  # 64 dims per core
DLH = DL // 2       # 32 dims per half-chunk
FL = DL * NE        # 1024 features per core
FH = FL // 2        # 512 features per half
MCH = S // 128      # 32 token chunks

_MASK11 = np.uint32(0xFFFFF000)  # keep 11 explicit mantissa bits
_QOFF = int(os.environ.get("KQOFF", "150"))
_EOFF = int(os.environ.get("KEOFF", "250"))
_ODEF = int(os.environ.get("KODEF", "4"))
_SOFF = int(os.environ.get("KSOFF", "150"))
_ALT = int(os.environ.get("KALT", "0"))

TRACE = False
_CACHE = {}


def _trunc11(a):
    a = np.ascontiguousarray(a, dtype=np.float32)
    return (a.view(np.uint32) & _MASK11).view(np.float32)


def _build():
    import concourse.mybir as mybir
    import concourse.tile as tile
    from concourse import bacc
    from contextlib import ExitStack

    F32 = mybir.dt.float32
    F32R = mybir.dt.float32r
    BF16 = mybir.dt.bfloat16
    AX = mybir.AxisListType
    OP = mybir.AluOpType
    ACTF = mybir.ActivationFunctionType

    from concourse import library_config

    nc = bacc.Bacc("TRN2", target_bir_lowering=False, debug=False,
                   num_devices=NCORES)

    uT = nc.dram_tensor("uT", [KU, S], F32R, kind="ExternalInput")
    wnnT = nc.dram_tensor("wnnT", [KU, FL], F32R, kind="ExternalInput")
    wnoT = nc.dram_tensor("wnoT", [KU, FL], F32R, kind="ExternalInput")
    weT = nc.dram_tensor("weT", [KU, FL], F32R, kind="ExternalInput")
    noise_c = nc.dram_tensor("noise_c", [S, FL], F32, kind="ExternalInput")
    cc = nc.dram_tensor("cc", [2, 3 * FL], F32R, kind="ExternalInput")
    out_c = nc.dram_tensor("out_c", [S, DL], F32, kind="ExternalOutput")

    with tile.TileContext(nc) as tc, ExitStack() as ctx:
        wpool = ctx.enter_context(tc.tile_pool(name="w", bufs=1))

        # per-(projection, half, k) weight tiles: each matmul depends only on
        # its own piece's DMA, so the k-loop paces with DMA arrival
        wdram = [wnnT, wnoT, weT]
        wt = [[[wpool.tile([128, FH], F32R, name=f"w{pi}_{hh}_{k}")
                for k in range(8)]
               for hh in range(2)] for pi in range(3)]

        def w_piece_dma(hh, k, boost=False):
            ksl = slice(k * 128, (k + 1) * 128)
            fs = slice(hh * FH, (hh + 1) * FH)
            from contextlib import nullcontext
            with tc.high_priority() if boost else nullcontext():
                for pi in range(3):
                    nc.sync.dma_start(wt[pi][hh][k][:], wdram[pi].ap()[ksl, fs])

        ccsb = wpool.tile([2, 3 * FL], F32R)
        nc.sync.dma_start(ccsb[:], cc.ap())

        nc.gpsimd.load_library(library_config.standard)

        onesf = wpool.tile([2, 128], F32)
        nc.vector.memset(onesf[:], 1.0)
        ones2 = wpool.tile([2, 128], F32R)
        nc.vector.tensor_copy(ones2[:], onesf[:])

        # broadcast the per-feature constants c (R-path + bias) across all
        # 128 partitions once, via a K=2 ones-matmul + Act drain; the main
        # loop then folds them in on Pool instead of PE bias matmuls
        cb = [[wpool.tile([128, FH], BF16 if pi == 2 else F32,
                          name=f"cb{pi}_{hh}") for hh in range(2)]
              for pi in range(3)]
        with tc.tile_pool(name="cbp", bufs=2, space="PSUM") as cbp:
            for pi in range(3):
                for hh in range(2):
                    cps = cbp.tile([128, FH], F32, tag="cps")
                    csl = slice(pi * FL + hh * FH, pi * FL + (hh + 1) * FH)
                    nc.tensor.matmul(cps[:], ones2[:], ccsb[:, csl],
                                     start=True, stop=True)
                    nc.scalar.copy(cb[pi][hh][:], cps[:])

        spool = ctx.enter_context(tc.tile_pool(name="stream", bufs=4))
        epool = ctx.enter_context(tc.tile_pool(name="epi", bufs=2))
        dpool = ctx.enter_context(tc.tile_pool(name="drain", bufs=3))
        mpsum = ctx.enter_context(tc.tile_pool(name="mps", bufs=3, space="PSUM"))
        mpsum_e = ctx.enter_context(tc.tile_pool(name="mpse", bufs=2, space="PSUM"))

        uT_r = uT.ap().rearrange("(kc p) t -> p kc t", p=128)

        # x/nz stream DMAs run at prefetch depth 3 (spool bufs=4) so each
        # iteration's inputs land well before its matmuls; out-DMAs are
        # issued two iterations late so their data-ready waits never
        # head-of-line-block the stream
        iters = [(hh, mm) for hh in range(2) for mm in range(MCH)]
        pending_outs = []
        stream = {}

        from contextlib import nullcontext

        def issue_stream(j, boost=True):
            if j >= len(iters):
                return
            hh, mm = iters[j]
            xs = spool.tile([128, 8, 128], F32R, tag="x")
            nzs = spool.tile([128, FH], F32, tag="nz")
            tsl_j = slice(mm * 128, (mm + 1) * 128)
            fsl_j = slice(hh * FH, (hh + 1) * FH)
            with tc.high_priority(offset=_SOFF) if boost else nullcontext():
                nc.sync.dma_start(xs[:], uT_r[:, :, tsl_j])
                nc.sync.dma_start(nzs[:], noise_c.ap()[tsl_j, fsl_j])
            stream[j] = (xs, nzs)

        issue_stream(0, boost=False)
        for k in range(8):
            w_piece_dma(0, k)
        issue_stream(1, boost=False)
        issue_stream(2, boost=False)

        for it, (half, m) in enumerate(iters):
            fsl = slice(half * FH, (half + 1) * FH)
            if True:
                tsl = slice(m * 128, (m + 1) * 128)
                issue_stream(it + 3)
                x_t, nz_t = stream.pop(it)
                while len(pending_outs) > _ODEF:
                    dst, src = pending_outs.pop(0)
                    nc.sync.dma_start(dst, src)
                if half == 0 and 1 <= m <= 8:
                    w_piece_dma(1, m - 1)
                h1p = mpsum.tile([128, FH], F32, tag="h1")
                h2p = mpsum.tile([128, FH], F32, tag="h2")
                ep = mpsum_e.tile([128, FH], F32, tag="e")

                # diagonal stagger: h1's chain starts one step after h2's,
                # e's two steps after, matching the spacing at which the Act
                # drains of the iteration-before-last release the PSUM banks
                for step in range(10):
                    if step < 8:
                        k = step
                        nc.tensor.matmul(h2p[:], x_t[:, k, :], wt[1][half][k][:],
                                         start=(k == 0), stop=(k == 7))
                    if 1 <= step <= 8:
                        k = step - 1
                        nc.tensor.matmul(h1p[:], x_t[:, k, :], wt[0][half][k][:],
                                         start=(k == 0), stop=(k == 7))
                    if step >= 2:
                        k = step - 2
                        nc.tensor.matmul(ep[:], x_t[:, k, :], wt[2][half][k][:],
                                         start=(k == 0), stop=(k == 7))

                # ---- epilogue: top-2 masked softmax dot e, mean over NE ----
                # Act drains PSUM; Pool folds c and the noise product;
                # DVE runs the f32 selection path + bf16 value path (2x mode)
                h2s = dpool.tile([128, FH], F32, tag="h2s")
                nc.scalar.copy(h2s[:], h2p[:])
                h1s = dpool.tile([128, FH], F32, tag="h1s")
                nc.scalar.copy(h1s[:], h1p[:])
                es0 = dpool.tile([128, FH], BF16, tag="es0")
                nc.scalar.copy(es0[:], ep[:])

                a_t = epool.tile([128, FH], F32, tag="a")
                (nc.vector if _ALT else nc.gpsimd).tensor_add(
                    a_t[:], h2s[:], cb[1][half][:])
                t_t = epool.tile([128, FH], F32, tag="t")
                nc.gpsimd.tensor_mul(t_t[:], a_t[:], nz_t[:])
                b_t = epool.tile([128, FH], F32, tag="b")
                nc.gpsimd.tensor_add(b_t[:], h1s[:], cb[0][half][:])
                es_t = epool.tile([128, FH], BF16, tag="es")
                nc.gpsimd.tensor_add(es_t[:], es0[:], cb[2][half][:])

                m_t = epool.tile([128, FH], F32, tag="m")
                nc.vector.tensor_add(m_t[:], t_t[:], b_t[:])
                q_t = epool.tile([128, FH], BF16, tag="q")
                # q/ev are not on the PSUM-release path: demote them behind
                # the next iteration's Act drains in the fixed schedule
                with tc.high_priority(offset=-_QOFF):
                    nc.scalar.activation(q_t[:], m_t[:], ACTF.Exp)

                mg = m_t[:].rearrange("p (d e) -> p d e", e=NE)
                v12 = epool.tile([128, 2 * DLH], F32, tag="v12")
                v1 = v12[:, :DLH]
                v2 = v12[:, DLH:]
                nc.vector.tensor_reduce(v1, mg, AX.X, op=OP.max)
                eq1 = epool.tile([128, FH], F32, tag="eq1")
                (nc.gpsimd if _ALT else nc.vector).tensor_tensor(
                    eq1[:].rearrange("p (d e) -> p d e", e=NE),
                    mg, v1.broadcast_to([128, DLH, NE]), OP.is_equal)
                m2 = epool.tile([128, FH], F32, tag="m2")
                nc.vector.scalar_tensor_tensor(m2[:], eq1[:], -1e30, m_t[:],
                                               OP.mult, OP.add)
                nc.vector.tensor_reduce(v2, m2[:].rearrange("p (d e) -> p d e", e=NE),
                                        AX.X, op=OP.max)
                mask = epool.tile([128, FH], BF16, tag="mask")
                (nc.gpsimd if _ALT else nc.vector).tensor_tensor(
                    mask[:].rearrange("p (d e) -> p d e", e=NE),
                    mg, v2.broadcast_to([128, DLH, NE]), OP.is_ge)
                p_t = epool.tile([128, FH], BF16, tag="p")
                nc.vector.tensor_mul(p_t[:], mask[:], q_t[:])
                num = epool.tile([128, FH], BF16, tag="num")
                nc.vector.tensor_mul(num[:], p_t[:], es_t[:])
                s_t = epool.tile([128, DLH], F32, tag="s")
                nc.vector.tensor_reduce(s_t[:], num[:].rearrange("p (d e) -> p d e", e=NE),
                                        AX.X, op=OP.add)
                ev12 = epool.tile([128, 2 * DLH], F32, tag="ev12")
                with tc.high_priority(offset=-_EOFF):
                    nc.scalar.activation(ev12[:], v12[:], ACTF.Exp)
                z_t = epool.tile([128, DLH], F32, tag="z")
                nc.vector.tensor_add(z_t[:], ev12[:, :DLH], ev12[:, DLH:])
                r_t = epool.tile([128, DLH], F32, tag="r")
                nc.vector.reciprocal_approx_fast(r_t[:], z_t[:])
                o_t = epool.tile([128, DLH], F32, tag="o")
                nc.vector.scalar_tensor_tensor(o_t[:], s_t[:], 1.0 / NE, r_t[:],
                                               OP.mult, OP.mult)
                pending_outs.append(
                    (out_c.ap()[tsl, half * DLH:(half + 1) * DLH], o_t[:]))

        for dst, src in pending_outs:
            nc.sync.dma_start(dst, src)

    nc.compile()
    return nc


def _get_program():
    if "nc" not in _CACHE:
        _CACHE["nc"] = _build()
    return _CACHE["nc"]


def kernel(h, us, ue, u, noise, W_nn, b_nn, W_no, b_no, W_E, b_E, W_r, b_r):
    from concourse.bass_utils import run_bass_kernel_spmd

    f32 = np.float32
    u2 = np.ascontiguousarray(np.asarray(u, dtype=f32).reshape(S, KU))
    uT = np.ascontiguousarray(u2.T)
    noise4 = np.asarray(noise, dtype=f32).reshape(S, DIM, NE)

    # R-path folded to per-feature constants on host, in fp64 (exact)
    hx64 = np.concatenate([np.asarray(h, dtype=f32).ravel(),
                           np.asarray(us, dtype=f32).ravel(),
                           np.asarray(ue, dtype=f32).ravel()]).astype(np.float64)
    R64 = hx64 @ np.asarray(W_r, dtype=f32).astype(np.float64).T \
        + np.asarray(b_r, dtype=f32).astype(np.float64)

    W_nn = np.asarray(W_nn, dtype=f32)
    W_no = np.asarray(W_no, dtype=f32)
    W_E = np.asarray(W_E, dtype=f32)
    biases = {"nn": np.asarray(b_nn, dtype=f32),
              "no": np.asarray(b_no, dtype=f32),
              "E": np.asarray(b_E, dtype=f32)}
    weights = {"nn": W_nn, "no": W_no, "E": W_E}

    in_maps = []
    for c in range(NCORES):
        fsl = slice(c * FL, (c + 1) * FL)
        cs = []
        for nm in ["nn", "no", "E"]:
            W_ = weights[nm]
            c64 = R64 @ W_[fsl, KU:].astype(np.float64).T \
                + biases[nm][fsl].astype(np.float64)
            cs.append(c64)
        c_all64 = np.concatenate(cs)
        ch = _trunc11(c_all64.astype(f32))
        cl = (c_all64 - ch.astype(np.float64)).astype(f32)
        im = {
            "uT": uT,
            "wnnT": np.ascontiguousarray(W_nn[fsl, :KU].T),
            "wnoT": np.ascontiguousarray(W_no[fsl, :KU].T),
            "weT": np.ascontiguousarray(W_E[fsl, :KU].T),
            "noise_c": np.ascontiguousarray(
                noise4[:, c * DL:(c + 1) * DL, :].reshape(S, FL)),
            "cc": np.ascontiguousarray(np.stack([ch, cl])),
        }
        in_maps.append(im)

    nc = _get_program()
    res = run_bass_kernel_spmd(nc, in_maps, core_ids=list(range(NCORES)),
                               trace=TRACE)
    _CACHE["last_results"] = res
    out = np.empty((1, S, DIM), dtype=f32)
    for c in range(NCORES):
        out[0, :, c * DL:(c + 1) * DL] = res.results[c]["out_c"]
    return out
